# revision 2
# baseline (speedup 1.0000x reference)
"""Multi-head attention (B=4, S=2048, D=1024, H=16) on 8 TRN2 NeuronCores. v2.

Sharding: 2D grid (batch x head-group), core c = g*4 + b.

v2 design vs baseline (544us):
- All matmul operands bf16 (host pre-converts x^T and weights to bf16;
  on-chip evictions write bf16). FWL-eligible 128-col stationaries.
- Scores for the head pair run CONCURRENTLY in 64x128 PE row tiles
  (T0: head0 on partitions 0:63, T8: head1 on 64:127) writing the two
  halves of one [128, 1024] PSUM region; ONE exp activation covers both.
- AV stationary padded to 128 cols with the ones/denominator column
  placed so head1's output lands on PSUM partitions 64:127 directly:
    h0 stationary = [V_h0(64) | ones | 0*63]   -> attn rows 0:63, den row 64
    h1 stationary = [ones | 0*63 | V_h1(64)]   -> den row 0, attn rows 64:127
  This kills the partition-shift DMA for h1 and makes every AV weight
  load a 128-col FWL load.
- ScalarE does ONLY the 256 exp instructions; all PSUM evictions and
  normalization run on DVE (tensor_scalar_add with per-partition bias).
- Software pipelining: pair p+1's projection slices and (for the last
  pair) the output projection are emitted interleaved into pair p's
  scalar-bound attention kt-loop so the PE never idles on phase edges.
PSUM budget: scores [128,1024] x2 (4 banks) + av h0/h1 (2) + proj/outproj
ping-pong (2) = 8. Transposes borrow the av-tag bank.
"""
import numpy as np

B, S, D, H, DH = 4, 2048, 1024, 16, 64
NCORES = 8
GCOLS = D // 2          # 512 cols per head-group core
NPAIRS = GCOLS // 128   # 4 head-pairs per core
NKT = S // 128          # 16 k-tiles
DC = D // 128           # 8 contraction chunks for projections
NQC = S // 512          # 4 q-chunks of 512

_COMPILED = None


def _build():
    import concourse.bass as bass
    import concourse.bacc as bacc
    import concourse.tile as tile
    from concourse import mybir
    from concourse.masks import make_identity
    from contextlib import ExitStack

    F32 = mybir.dt.float32
    BF16 = mybir.dt.bfloat16
    EXP = mybir.ActivationFunctionType.Exp

    nc = bacc.Bacc("TRN2", target_bir_lowering=False, debug=False)
    xT = nc.dram_tensor("xT", [D, S], BF16, kind="ExternalInput").ap()
    wq = nc.dram_tensor("wq", [D, GCOLS], BF16, kind="ExternalInput").ap()
    wk = nc.dram_tensor("wk", [D, GCOLS], BF16, kind="ExternalInput").ap()
    wv = nc.dram_tensor("wv", [D, GCOLS], BF16, kind="ExternalInput").ap()
    wo = nc.dram_tensor("wo", [GCOLS, D], BF16, kind="ExternalInput").ap()
    bq = nc.dram_tensor("bq", [GCOLS], F32, kind="ExternalInput").ap()
    bk = nc.dram_tensor("bk", [GCOLS], F32, kind="ExternalInput").ap()
    bv = nc.dram_tensor("bv", [GCOLS], F32, kind="ExternalInput").ap()
    out = nc.dram_tensor("out", [S, D], F32, kind="ExternalOutput").ap()

    with tile.TileContext(nc) as tc, ExitStack() as st:
        const = st.enter_context(tc.tile_pool(name="const", bufs=1))
        persist = st.enter_context(tc.tile_pool(name="persist", bufs=1))
        qkv = st.enter_context(tc.tile_pool(name="qkv", bufs=1))
        vts = st.enter_context(tc.tile_pool(name="vts", bufs=1))
        espool = st.enter_context(tc.tile_pool(name="es", bufs=1))
        small = st.enter_context(tc.tile_pool(name="small", bufs=1))
        osb = st.enter_context(tc.tile_pool(name="osb", bufs=1))
        psum = st.enter_context(tc.tile_pool(name="ps", bufs=1, space="PSUM"))

        # ---------------- constants / weights / x^T ----------------
        idf = const.tile([128, 128], F32)
        make_identity(nc, idf)
        # ones-row tile for PE den-broadcast: row 0 and row 64 are ones so
        # slices [0:64] / [64:128] each have local row 0 = ones
        ones_t = const.tile([128, 128], BF16, name="ones_t", tag="ones_t")
        nc.vector.memset(ones_t, 0.0)
        nc.vector.memset(ones_t[0:1, :], 1.0)
        nc.vector.memset(ones_t[64:65, :], 1.0)
        bias_sb = {}
        for nm, b_ap in (("q", bq), ("k", bk), ("v", bv)):
            t = const.tile([128, NPAIRS], F32, name=f"b{nm}", tag=f"b{nm}")
            nc.sync.dma_start(out=t, in_=b_ap.rearrange("(p r) -> r p", r=128))
            bias_sb[nm] = t

        xr = persist.tile([128, DC, S], BF16, name="xr", tag="xr")
        xT_d = xT.rearrange("(dc p) n -> p dc n", p=128)
        for dc in range(DC):
            [nc.sync, nc.scalar][dc % 2].dma_start(
                out=xr[:, dc, :], in_=xT_d[:, dc, :])
        w_sb = {}
        for nm, w_ap in (("q", wq), ("k", wk), ("v", wv)):
            t = persist.tile([128, DC, GCOLS], BF16, name=f"w{nm}",
                             tag=f"w{nm}")
            nc.sync.dma_start(out=t, in_=w_ap.rearrange("(dc p) m -> p dc m",
                                                        p=128))
            w_sb[nm] = t
        wo_sb = persist.tile([128, NPAIRS, D], BF16, name="wo", tag="wo")
        nc.sync.dma_start(out=wo_sb, in_=wo.rearrange("(p r) n -> r p n",
                                                      r=128))

        # v_sb: [128, 2, NKT, 256] bf16, double-buffered by pair parity;
        # per pair: cols 0:64 = V_h0, 64 = ones, 65:128 = 0,
        #           128 = ones, 129:192 = 0, 192:256 = V_h1.
        v_sb = persist.tile([128, 2, NKT, 256], BF16, name="v_sb",
                            tag="v_sb")
        nc.vector.memset(v_sb, 0.0)
        nc.vector.memset(v_sb[:, :, :, 64:65], 1.0)
        nc.vector.memset(v_sb[:, :, :, 128:129], 1.0)

        # attnT tiles (out-proj stationaries), bf16 [128, S] per pair
        attnT = [persist.tile([128, S], BF16, name=f"attnT{p}",
                              tag=f"attnT{p}") for p in range(NPAIRS)]

        # double-buffered per-pair projection outputs
        def qk_tile(nm, p):
            return qkv.tile([128, S], BF16, name=f"{nm}t", tag=f"{nm}t",
                            bufs=2)

        # ---------------- PSUM tags ----------------
        # 8 banks: sc 2x[128,1024] (4; also serves projections, V-transposes
        # and the output projection) + av 4x[128,512] (4: per-head lo/hi
        # contraction halves so AV runs in the same 64x128 tiled mode as
        # the scores -- the PE never changes tiling mode inside the
        # attention loop; mode switches cost ~240ns each).
        def sc_ps():
            return psum.tile([128, 1024], F32, name="sc", tag="sc", bufs=2)

        def av_ps():
            return psum.tile([128, 512], F32, name="av", tag="av", bufs=4)

        def proj_ps():
            t = sc_ps()
            return t[:, 0:512]

        def proj_ps2():
            return sc_ps()

        def mm(out_, l, r, **kw):
            nc.tensor.matmul(out_, l, r, skip_group_check=True, **kw)

        # ---------------- warmup (PE ramp) ----------------
        zb = const.tile([128, 512], BF16)
        nc.vector.memset(zb, 0.0)
        for i in range(24):
            w_ps = proj_ps()
            mm(w_ps, zb[:, 0:128], zb, start=True, stop=True)

        # ---------------- emission helpers ----------------
        cur = {}   # current pair's projection tiles

        def proj_slots(p, qc, which):
            """Projection nt-chunk for pair p as a list of small emission
            slots (each <= ~1.2us of PE work) so the scalar engine's 1-exp
            lookahead never runs dry while a slot occupies the PE queue."""
            nsl = slice(qc * 512, (qc + 1) * 512)
            state = {}

            def slot_a():
                state["ps"] = proj_ps()
                for dc in range(4):
                    mm(state["ps"], w_sb[which][:, dc, p * 128:(p + 1) * 128],
                       xr[:, dc, nsl], start=(dc == 0), stop=False)

            def slot_b():
                mm_ps = state["ps"]
                for dc in range(4, DC):
                    mm(mm_ps, w_sb[which][:, dc, p * 128:(p + 1) * 128],
                       xr[:, dc, nsl], start=False, stop=(dc == DC - 1))
                if which in ("q", "k"):
                    t = cur[(which, p)]
                    nc.vector.tensor_scalar_add(t[:, nsl], mm_ps,
                                                bias_sb[which][:, p:p + 1])
                else:
                    vt = vts.tile([128, 512], F32, name="vt", tag="vt",
                                  bufs=2)
                    nc.vector.tensor_scalar_add(vt, mm_ps,
                                                bias_sb["v"][:, p:p + 1])
                    state["vt"] = vt

            def slot_c():
                vt = state["vt"]
                tr = proj_ps()
                for j in range(4):
                    mm(tr[:, j * 128:(j + 1) * 128],
                       vt[:, j * 128:(j + 1) * 128], idf, is_transpose=True,
                       start=True, stop=True)
                trr = tr.rearrange("p (a b) -> p a b", b=128)
                kts = slice(qc * 4, (qc + 1) * 4)
                nc.vector.tensor_copy(v_sb[:, p % 2, kts, 0:64],
                                      trr[:, :, 0:64])
                nc.vector.tensor_copy(v_sb[:, p % 2, kts, 192:256],
                                      trr[:, :, 64:128])

            if which == "v":
                return [slot_a, slot_b, slot_c]
            return [slot_a, slot_b]

        def emit_proj_piece(p, qc, which):
            for s in proj_slots(p, qc, which):
                s()

        def start_pair(p):
            cur[("q", p)] = qk_tile("q", p)
            cur[("k", p)] = qk_tile("k", p)

        def emit_attention(p, qc, interleave, points=(1, 2, 4, 6, 8, 10,
                                                     12, 14)):
            """Attention for (pair, q-chunk); interleave = list of callables
            invoked between kt iterations (projection pieces etc.).
            Software-pipelined: scores(kt+1) is emitted BEFORE AV(kt) so the
            in-order PE queue never stalls behind the exp dependency."""
            qt, kt_t = cur[("q", p)], cur[("k", p)]
            vb = v_sb[:, p % 2]
            qsl = slice(qc * 512, (qc + 1) * 512)
            av = [av_ps() for _ in range(4)]  # h0lo h0hi h1lo h1hi
            inter = list(interleave)
            es_t = [None] * NKT

            def scores(kt):
                sc = sc_ps()
                ksl = slice(kt * 128, (kt + 1) * 128)
                mm(sc[:, 0:512], kt_t[0:64, ksl], qt[0:64, qsl],
                   start=True, stop=True)
                mm(sc[:, 512:1024], kt_t[64:128, ksl], qt[64:128, qsl],
                   start=True, stop=True)
                if kt in (3, 7, 11, 15):
                    # Schraudolph exp on DVE: bf16 bit pattern of exp(s/8)
                    # as one affine op into uint16, reinterpreted as bf16.
                    # Keeps ScalarE strictly faster than the PE so the PE
                    # never idles (idle >100ns halves the PE clock).
                    esu = espool.tile([128, 1024], mybir.dt.uint16,
                                      name="esu", tag="esu", bufs=2)
                    nc.vector.tensor_scalar(
                        out=esu, in0=sc, scalar1=23.08312065,
                        scalar2=16250.4093, op0=mybir.AluOpType.mult,
                        op1=mybir.AluOpType.add)
                    es_t[kt] = esu.bitcast(BF16)
                else:
                    es = espool.tile([128, 1024], BF16, name="es", tag="es",
                                     bufs=3)
                    nc.scalar.activation(es, sc, EXP, scale=0.125)
                    es_t[kt] = es

            scores(0)
            for kt in range(NKT):
                if kt + 1 < NKT:
                    scores(kt + 1)
                es = es_t[kt]
                st, sp = (kt == 0), (kt == NKT - 1)
                mm(av[0], vb[0:64, kt, 0:128], es[0:64, 0:512],
                   start=st, stop=sp)
                mm(av[1], vb[64:128, kt, 0:128], es[64:128, 0:512],
                   start=st, stop=sp)
                mm(av[2], vb[0:64, kt, 128:256], es[0:64, 512:1024],
                   start=st, stop=sp)
                mm(av[3], vb[64:128, kt, 128:256], es[64:128, 512:1024],
                   start=st, stop=sp)
                # one small slot per point, never adjacent to the window
                # boundary (a slot at kt=15 would sit in the sc-tag
                # rotation directly ahead of the next window's scores)
                if inter and kt in points:
                    inter.pop(0)()
            # merge contraction halves + normalize
            # h0: attn rows 0:63, den row 64 ; h1: attn rows 64:127, den row 0
            avh0 = small.tile([128, 512], BF16, name="avh0", tag="avh0",
                              bufs=2)
            avh1 = small.tile([128, 512], BF16, name="avh1", tag="avh1",
                              bufs=2)
            t0 = small.tile([128, 512], F32, name="tlo", tag="tlo", bufs=2)
            t1 = small.tile([128, 512], F32, name="thi", tag="thi", bufs=2)
            nc.vector.tensor_copy(t0, av[0])
            nc.vector.tensor_add(avh0, av[1], t0)
            nc.vector.tensor_copy(t1, av[2])
            nc.vector.tensor_add(avh1, av[3], t1)
            # den broadcast THROUGH THE PE: a ones-row stationary matmul in
            # the same 64x128 tile mode replicates the den row across all
            # 128 psum partitions -- no DMA, no queue contention, no mode
            # switch. bc matmuls borrow freed av-tag banks.
            bc0 = av_ps()
            bc1 = av_ps()
            mm(bc0, ones_t[64:128, :], avh0[64:128, :], start=True,
               stop=True)
            mm(bc1, ones_t[0:64, :], avh1[0:64, :], start=True, stop=True)
            # reciprocal_approx_fast requires a full-128-partition SBUF
            # source on HW (PSUM or partial-partition sources NaN silently)
            bcs = small.tile([128, 512], F32, name="bcs", tag="bcs", bufs=2)
            nc.vector.tensor_copy(bcs[0:64, :], bc0[0:64, :])
            nc.vector.tensor_copy(bcs[64:128, :], bc1[64:128, :])
            rec = small.tile([128, 512], F32, name="rec", tag="rec", bufs=2)
            nc.vector.reciprocal_approx_fast(out=rec, in_=bcs)
            nc.vector.tensor_mul(attnT[p][0:64, qsl], avh0[0:64, :],
                                 rec[0:64, :])
            nc.vector.tensor_mul(attnT[p][64:128, qsl], avh1[64:128, :],
                                 rec[64:128, :])

        def outproj_slots(qcb):
            qbs = slice(qcb * 128, (qcb + 1) * 128)
            state = {}

            def slot(nt):
                if nt == 0:
                    state["ps"] = proj_ps2()
                ops = state["ps"]
                for p in range(NPAIRS):
                    mm(ops[:, nt * 512:(nt + 1) * 512], attnT[p][:, qbs],
                       wo_sb[:, p, nt * 512:(nt + 1) * 512],
                       start=(p == 0), stop=(p == NPAIRS - 1))
                if nt == 1:
                    o_sb = osb.tile([128, 1024], F32, name="o", tag="o",
                                    bufs=3)
                    nc.vector.tensor_copy(o_sb, ops)
                    nc.sync.dma_start(out=out[qbs, :], in_=o_sb)

            return [lambda: slot(0), lambda: slot(1)]

        def outproj_piece(qcb):
            for s in outproj_slots(qcb):
                s()

        # ---------------- pipeline ----------------
        # Short prologue: only pair-0 chunk-0 projections plus k1 run before
        # the first attention window; the remaining pair-0 chunks and ALL of
        # pair-1's projections are slotted into pair-0's windows with
        # deadline-aware points (chunk j must be evicted before the scores/
        # AV instruction that reads it is EMITTED, or the in-order PE queue
        # deadlocks).
        def vm(p, qc):
            s = proj_slots(p, qc, "v")
            return [lambda a=s[0], b=s[1]: (a(), b()), s[2]]

        start_pair(0)
        for which in ("q", "k", "v"):
            emit_proj_piece(0, 0, which)
        emit_proj_piece(0, 1, "k")
        start_pair(1)
        # deadlines: k-chunk j before scores(4j) emission (kt 4j-1),
        # v-chunk j before AV(4j) emission (kt 4j), q-chunk qc before
        # window (0,qc) emission.
        sched = {
            (0, 0): (proj_slots(0, 1, "v") + proj_slots(0, 2, "k")
                     + vm(0, 2) + proj_slots(0, 3, "k") + vm(0, 3)
                     + proj_slots(0, 1, "q"),
                     (1, 2, 3, 4, 5, 6, 7, 8, 9, 10, 11, 12, 13)),
            (0, 1): (proj_slots(0, 2, "q") + proj_slots(1, 0, "q")
                     + proj_slots(1, 0, "k") + vm(1, 0),
                     (1, 2, 3, 4, 5, 6, 8, 10, 12)),
            (0, 2): (proj_slots(0, 3, "q") + proj_slots(1, 1, "q")
                     + proj_slots(1, 1, "k") + vm(1, 1),
                     (1, 2, 3, 4, 5, 6, 8, 10, 12)),
            (0, 3): (proj_slots(1, 2, "q") + proj_slots(1, 2, "k")
                     + vm(1, 2) + proj_slots(1, 3, "q")
                     + proj_slots(1, 3, "k") + vm(1, 3),
                     (1, 2, 3, 4, 5, 6, 7, 8, 9, 10, 12, 14)),
        }
        for p in range(NPAIRS):
            if p + 1 < NPAIRS:
                start_pair(p + 1) if p > 0 else None
            for qc in range(NQC):
                if (p, qc) in sched:
                    inter, points = sched[(p, qc)]
                elif p + 1 < NPAIRS:
                    inter = []
                    for w in ("q", "k", "v"):
                        inter += proj_slots(p + 1, qc, w)
                    points = (1, 2, 4, 6, 8, 10, 12, 14)
                else:
                    inter = []
                    if qc > 0:
                        for s in range(4):
                            inter += outproj_slots((qc - 1) * 4 + s)
                    points = (1, 2, 4, 6, 8, 10, 12, 14)
                emit_attention(p, qc, inter, points)
        for s in range(4):
            outproj_piece((NQC - 1) * 4 + s)

    nc.compile()
    return nc


def _get_compiled():
    global _COMPILED
    if _COMPILED is None:
        _COMPILED = _build()
    return _COMPILED


def make_in_maps(**inputs):
    import ml_dtypes
    bf = ml_dtypes.bfloat16
    x = np.asarray(inputs["inputs"], np.float32)
    xTb = [np.ascontiguousarray(x[b].T.astype(bf)) for b in range(B)]
    gs = {}
    for nm in ("Wq", "Wk", "Wv", "Wo", "bq", "bk", "bv"):
        a = np.asarray(inputs[nm], np.float32)
        for g in range(2):
            sl = slice(g * GCOLS, (g + 1) * GCOLS)
            if nm == "Wo":
                gs[(nm, g)] = np.ascontiguousarray(a[sl, :].astype(bf))
            elif nm.startswith("W"):
                gs[(nm, g)] = np.ascontiguousarray(a[:, sl].astype(bf))
            else:
                gs[(nm, g)] = np.ascontiguousarray(a[sl])
    in_maps = []
    for c in range(NCORES):
        g, b = c // B, c % B
        in_maps.append({
            "xT": xTb[b],
            "wq": gs[("Wq", g)], "wk": gs[("Wk", g)],
            "wv": gs[("Wv", g)], "wo": gs[("Wo", g)],
            "bq": gs[("bq", g)], "bk": gs[("bk", g)],
            "bv": gs[("bv", g)],
        })
    return in_maps


def combine(results, bo):
    out = np.empty((B, S, D), np.float32)
    bo = np.asarray(bo, np.float32)
    for b in range(B):
        out[b] = results[b]["out"] + results[B + b]["out"] + bo
    return out


def kernel(**inputs):
    from concourse import bass_utils
    nc = _get_compiled()
    in_maps = make_in_maps(**inputs)
    res = bass_utils.run_bass_kernel_spmd(
        nc, in_maps, core_ids=list(range(NCORES)))
    return combine(res.results, inputs["bo"])


# revision 3
# speedup vs baseline: 1.0651x; 1.0651x over previous
"""Multi-head attention (B=4, S=2048, D=1024, H=16) on 8 TRN2 NeuronCores. v2.

Sharding: 2D grid (batch x head-group), core c = g*4 + b.

v2 design vs baseline (544us):
- All matmul operands bf16 (host pre-converts x^T and weights to bf16;
  on-chip evictions write bf16). FWL-eligible 128-col stationaries.
- Scores for the head pair run CONCURRENTLY in 64x128 PE row tiles
  (T0: head0 on partitions 0:63, T8: head1 on 64:127) writing the two
  halves of one [128, 1024] PSUM region; ONE exp activation covers both.
- AV stationary padded to 128 cols with the ones/denominator column
  placed so head1's output lands on PSUM partitions 64:127 directly:
    h0 stationary = [V_h0(64) | ones | 0*63]   -> attn rows 0:63, den row 64
    h1 stationary = [ones | 0*63 | V_h1(64)]   -> den row 0, attn rows 64:127
  This kills the partition-shift DMA for h1 and makes every AV weight
  load a 128-col FWL load.
- ScalarE does ONLY the 256 exp instructions; all PSUM evictions and
  normalization run on DVE (tensor_scalar_add with per-partition bias).
- Software pipelining: pair p+1's projection slices and (for the last
  pair) the output projection are emitted interleaved into pair p's
  scalar-bound attention kt-loop so the PE never idles on phase edges.
PSUM budget: scores [128,1024] x2 (4 banks) + av h0/h1 (2) + proj/outproj
ping-pong (2) = 8. Transposes borrow the av-tag bank.
"""
import numpy as np

B, S, D, H, DH = 4, 2048, 1024, 16, 64
NCORES = 8
GCOLS = D // 2          # 512 cols per head-group core
NPAIRS = GCOLS // 128   # 4 head-pairs per core
NKT = S // 128          # 16 k-tiles
DC = D // 128           # 8 contraction chunks for projections
NQC = S // 512          # 4 q-chunks of 512

_COMPILED = None


def _build():
    import concourse.bass as bass
    import concourse.bacc as bacc
    import concourse.tile as tile
    from concourse import mybir
    from concourse.masks import make_identity
    from contextlib import ExitStack

    F32 = mybir.dt.float32
    BF16 = mybir.dt.bfloat16
    EXP = mybir.ActivationFunctionType.Exp

    nc = bacc.Bacc("TRN2", target_bir_lowering=False, debug=False)
    xT = nc.dram_tensor("xT", [D, S], BF16, kind="ExternalInput").ap()
    wq = nc.dram_tensor("wq", [D, GCOLS], BF16, kind="ExternalInput").ap()
    wk = nc.dram_tensor("wk", [D, GCOLS], BF16, kind="ExternalInput").ap()
    wv = nc.dram_tensor("wv", [D, GCOLS], BF16, kind="ExternalInput").ap()
    wo = nc.dram_tensor("wo", [GCOLS, D], BF16, kind="ExternalInput").ap()
    bq = nc.dram_tensor("bq", [GCOLS], F32, kind="ExternalInput").ap()
    bk = nc.dram_tensor("bk", [GCOLS], F32, kind="ExternalInput").ap()
    bv = nc.dram_tensor("bv", [GCOLS], F32, kind="ExternalInput").ap()
    out = nc.dram_tensor("out", [S, D], F32, kind="ExternalOutput").ap()

    with tile.TileContext(nc) as tc, ExitStack() as st:
        const = st.enter_context(tc.tile_pool(name="const", bufs=1))
        persist = st.enter_context(tc.tile_pool(name="persist", bufs=1))
        qkv = st.enter_context(tc.tile_pool(name="qkv", bufs=1))
        vts = st.enter_context(tc.tile_pool(name="vts", bufs=1))
        espool = st.enter_context(tc.tile_pool(name="es", bufs=1))
        small = st.enter_context(tc.tile_pool(name="small", bufs=1))
        osb = st.enter_context(tc.tile_pool(name="osb", bufs=1))
        psum = st.enter_context(tc.tile_pool(name="ps", bufs=1, space="PSUM"))

        # ---------------- constants / weights / x^T ----------------
        idf = const.tile([128, 128], F32)
        make_identity(nc, idf)
        # ones-row tile for PE den-broadcast: row 0 and row 64 are ones so
        # slices [0:64] / [64:128] each have local row 0 = ones
        ones_t = const.tile([128, 128], BF16, name="ones_t", tag="ones_t")
        nc.vector.memset(ones_t, 0.0)
        nc.vector.memset(ones_t[0:1, :], 1.0)
        nc.vector.memset(ones_t[64:65, :], 1.0)
        bias_sb = {}
        for nm, b_ap in (("q", bq), ("k", bk), ("v", bv)):
            t = const.tile([128, NPAIRS], F32, name=f"b{nm}", tag=f"b{nm}")
            nc.sync.dma_start(out=t, in_=b_ap.rearrange("(p r) -> r p", r=128))
            bias_sb[nm] = t

        xr = persist.tile([128, DC, S], BF16, name="xr", tag="xr")
        xT_d = xT.rearrange("(dc p) n -> p dc n", p=128)
        for dc in range(DC):
            [nc.sync, nc.scalar][dc % 2].dma_start(
                out=xr[:, dc, :], in_=xT_d[:, dc, :])
        w_sb = {}
        for nm, w_ap in (("q", wq), ("k", wk), ("v", wv)):
            t = persist.tile([128, DC, GCOLS], BF16, name=f"w{nm}",
                             tag=f"w{nm}")
            nc.sync.dma_start(out=t, in_=w_ap.rearrange("(dc p) m -> p dc m",
                                                        p=128))
            w_sb[nm] = t
        wo_sb = persist.tile([128, NPAIRS, D], BF16, name="wo", tag="wo")
        nc.sync.dma_start(out=wo_sb, in_=wo.rearrange("(p r) n -> r p n",
                                                      r=128))

        # v_sb: [128, 2, NKT, 256] bf16, double-buffered by pair parity;
        # per pair: cols 0:64 = V_h0, 64 = ones, 65:128 = 0,
        #           128 = ones, 129:192 = 0, 192:256 = V_h1.
        v_sb = persist.tile([128, 2, NKT, 256], BF16, name="v_sb",
                            tag="v_sb")
        nc.vector.memset(v_sb, 0.0)
        nc.vector.memset(v_sb[:, :, :, 64:65], 1.0)
        nc.vector.memset(v_sb[:, :, :, 128:129], 1.0)

        # attnT tiles (out-proj stationaries), bf16 [128, S] per pair
        attnT = [persist.tile([128, S], BF16, name=f"attnT{p}",
                              tag=f"attnT{p}") for p in range(NPAIRS)]

        # double-buffered per-pair projection outputs
        def qk_tile(nm, p):
            return qkv.tile([128, S], BF16, name=f"{nm}t", tag=f"{nm}t",
                            bufs=2)

        # ---------------- PSUM tags ----------------
        # 8 banks: sc 2x[128,1024] (4; also serves projections, V-transposes
        # and the output projection) + av 4x[128,512] (4: per-head lo/hi
        # contraction halves so AV runs in the same 64x128 tiled mode as
        # the scores -- the PE never changes tiling mode inside the
        # attention loop; mode switches cost ~240ns each).
        def sc_ps():
            return psum.tile([128, 1024], F32, name="sc", tag="sc", bufs=2)

        def av_ps():
            return psum.tile([128, 512], F32, name="av", tag="av", bufs=4)

        def proj_ps():
            t = sc_ps()
            return t[:, 0:512]

        def proj_ps2():
            return sc_ps()

        def mm(out_, l, r, **kw):
            nc.tensor.matmul(out_, l, r, skip_group_check=True, **kw)

        # ---------------- warmup (PE ramp) ----------------
        zb = const.tile([128, 512], BF16)
        nc.vector.memset(zb, 0.0)
        for i in range(24):
            w_ps = proj_ps()
            mm(w_ps, zb[:, 0:128], zb, start=True, stop=True)

        # ---------------- emission helpers ----------------
        cur = {}   # current pair's projection tiles

        def proj_slots(p, qc, which):
            """Projection nt-chunk for pair p as a list of small emission
            slots (each <= ~1.2us of PE work) so the scalar engine's 1-exp
            lookahead never runs dry while a slot occupies the PE queue."""
            nsl = slice(qc * 512, (qc + 1) * 512)
            state = {}

            def slot_a():
                state["ps"] = proj_ps()
                for dc in range(4):
                    mm(state["ps"], w_sb[which][:, dc, p * 128:(p + 1) * 128],
                       xr[:, dc, nsl], start=(dc == 0), stop=False)

            def slot_b():
                mm_ps = state["ps"]
                for dc in range(4, DC):
                    mm(mm_ps, w_sb[which][:, dc, p * 128:(p + 1) * 128],
                       xr[:, dc, nsl], start=False, stop=(dc == DC - 1))
                if which in ("q", "k"):
                    t = cur[(which, p)]
                    nc.vector.tensor_scalar_add(t[:, nsl], mm_ps,
                                                bias_sb[which][:, p:p + 1])
                else:
                    vt = vts.tile([128, 512], F32, name="vt", tag="vt",
                                  bufs=2)
                    nc.vector.tensor_scalar_add(vt, mm_ps,
                                                bias_sb["v"][:, p:p + 1])
                    state["vt"] = vt

            def slot_c():
                vt = state["vt"]
                tr = proj_ps()
                for j in range(4):
                    mm(tr[:, j * 128:(j + 1) * 128],
                       vt[:, j * 128:(j + 1) * 128], idf, is_transpose=True,
                       start=True, stop=True)
                trr = tr.rearrange("p (a b) -> p a b", b=128)
                kts = slice(qc * 4, (qc + 1) * 4)
                nc.vector.tensor_copy(v_sb[:, p % 2, kts, 0:64],
                                      trr[:, :, 0:64])
                nc.vector.tensor_copy(v_sb[:, p % 2, kts, 192:256],
                                      trr[:, :, 64:128])

            if which == "v":
                return [slot_a, slot_b, slot_c]
            return [slot_a, slot_b]

        def emit_proj_piece(p, qc, which):
            for s in proj_slots(p, qc, which):
                s()

        def start_pair(p):
            cur[("q", p)] = qk_tile("q", p)
            cur[("k", p)] = qk_tile("k", p)

        def emit_attention(p, qc, interleave, points=(1, 2, 4, 6, 8, 10,
                                                     12, 14)):
            """Attention for (pair, q-chunk); interleave = list of callables
            invoked between kt iterations (projection pieces etc.).
            Software-pipelined: scores(kt+1) is emitted BEFORE AV(kt) so the
            in-order PE queue never stalls behind the exp dependency."""
            qt, kt_t = cur[("q", p)], cur[("k", p)]
            vb = v_sb[:, p % 2]
            qsl = slice(qc * 512, (qc + 1) * 512)
            av = [av_ps() for _ in range(4)]  # h0lo h0hi h1lo h1hi
            inter = list(interleave)
            es_t = [None] * NKT

            def scores(kt):
                sc = sc_ps()
                ksl = slice(kt * 128, (kt + 1) * 128)
                mm(sc[:, 0:512], kt_t[0:64, ksl], qt[0:64, qsl],
                   start=True, stop=True)
                mm(sc[:, 512:1024], kt_t[64:128, ksl], qt[64:128, qsl],
                   start=True, stop=True)
                if kt in (6, 9, 12, 15):
                    # Schraudolph exp on DVE: bf16 bit pattern of exp(s/8)
                    # as one affine op into uint16, reinterpreted as bf16.
                    # Keeps ScalarE strictly faster than the PE so the PE
                    # never idles (idle >100ns halves the PE clock).
                    esu = espool.tile([128, 1024], mybir.dt.uint16,
                                      name="esu", tag="esu", bufs=2)
                    nc.vector.tensor_scalar(
                        out=esu, in0=sc, scalar1=23.08312065,
                        scalar2=16250.4093, op0=mybir.AluOpType.mult,
                        op1=mybir.AluOpType.add)
                    es_t[kt] = esu.bitcast(BF16)
                else:
                    es = espool.tile([128, 1024], BF16, name="es", tag="es",
                                     bufs=3)
                    nc.scalar.activation(es, sc, EXP, scale=0.125)
                    es_t[kt] = es

            scores(0)
            for kt in range(NKT):
                if kt + 1 < NKT:
                    scores(kt + 1)
                es = es_t[kt]
                st, sp = (kt == 0), (kt == NKT - 1)
                mm(av[0], vb[0:64, kt, 0:128], es[0:64, 0:512],
                   start=st, stop=sp)
                mm(av[1], vb[64:128, kt, 0:128], es[64:128, 0:512],
                   start=st, stop=sp)
                mm(av[2], vb[0:64, kt, 128:256], es[0:64, 512:1024],
                   start=st, stop=sp)
                mm(av[3], vb[64:128, kt, 128:256], es[64:128, 512:1024],
                   start=st, stop=sp)
                # one small slot per point, never adjacent to the window
                # boundary (a slot at kt=15 would sit in the sc-tag
                # rotation directly ahead of the next window's scores)
                if inter and kt in points:
                    inter.pop(0)()
            # merge contraction halves + normalize
            # h0: attn rows 0:63, den row 64 ; h1: attn rows 64:127, den row 0
            avh0 = small.tile([128, 512], BF16, name="avh0", tag="avh0",
                              bufs=2)
            avh1 = small.tile([128, 512], BF16, name="avh1", tag="avh1",
                              bufs=2)
            t0 = small.tile([128, 512], F32, name="tlo", tag="tlo", bufs=2)
            t1 = small.tile([128, 512], F32, name="thi", tag="thi", bufs=2)
            nc.vector.tensor_copy(t0, av[0])
            nc.vector.tensor_add(avh0, av[1], t0)
            nc.vector.tensor_copy(t1, av[2])
            nc.vector.tensor_add(avh1, av[3], t1)
            # den broadcast THROUGH THE PE: a ones-row stationary matmul in
            # the same 64x128 tile mode replicates the den row across all
            # 128 psum partitions -- no DMA, no queue contention, no mode
            # switch. bc matmuls borrow freed av-tag banks.
            bc0 = av_ps()
            bc1 = av_ps()
            mm(bc0, ones_t[64:128, :], avh0[64:128, :], start=True,
               stop=True)
            mm(bc1, ones_t[0:64, :], avh1[0:64, :], start=True, stop=True)
            # reciprocal_approx_fast requires a full-128-partition SBUF
            # source on HW (PSUM or partial-partition sources NaN silently)
            bcs = small.tile([128, 512], F32, name="bcs", tag="bcs", bufs=2)
            nc.vector.tensor_copy(bcs[0:64, :], bc0[0:64, :])
            nc.vector.tensor_copy(bcs[64:128, :], bc1[64:128, :])
            rec = small.tile([128, 512], F32, name="rec", tag="rec", bufs=2)
            nc.vector.reciprocal_approx_fast(out=rec, in_=bcs)
            nc.vector.tensor_mul(attnT[p][0:64, qsl], avh0[0:64, :],
                                 rec[0:64, :])
            nc.vector.tensor_mul(attnT[p][64:128, qsl], avh1[64:128, :],
                                 rec[64:128, :])

        def outproj_slots(qcb):
            qbs = slice(qcb * 128, (qcb + 1) * 128)
            state = {}

            def slot(nt):
                if nt == 0:
                    state["ps"] = proj_ps2()
                ops = state["ps"]
                for p in range(NPAIRS):
                    mm(ops[:, nt * 512:(nt + 1) * 512], attnT[p][:, qbs],
                       wo_sb[:, p, nt * 512:(nt + 1) * 512],
                       start=(p == 0), stop=(p == NPAIRS - 1))
                if nt == 1:
                    o_sb = osb.tile([128, 1024], F32, name="o", tag="o",
                                    bufs=3)
                    nc.vector.tensor_copy(o_sb, ops)
                    nc.sync.dma_start(out=out[qbs, :], in_=o_sb)

            return [lambda: slot(0), lambda: slot(1)]

        def outproj_piece(qcb):
            for s in outproj_slots(qcb):
                s()

        # ---------------- pipeline ----------------
        # Short prologue: only pair-0 chunk-0 projections plus k1 run before
        # the first attention window; the remaining pair-0 chunks and ALL of
        # pair-1's projections are slotted into pair-0's windows with
        # deadline-aware points (chunk j must be evicted before the scores/
        # AV instruction that reads it is EMITTED, or the in-order PE queue
        # deadlocks).
        def vm(p, qc):
            s = proj_slots(p, qc, "v")
            return [lambda a=s[0], b=s[1]: (a(), b()), s[2]]

        start_pair(0)
        for which in ("q", "k", "v"):
            emit_proj_piece(0, 0, which)
        emit_proj_piece(0, 1, "k")
        start_pair(1)
        # deadlines: k-chunk j before scores(4j) emission (kt 4j-1),
        # v-chunk j before AV(4j) emission (kt 4j), q-chunk qc before
        # window (0,qc) emission.
        sched = {
            (0, 0): (proj_slots(0, 1, "v") + proj_slots(0, 2, "k")
                     + vm(0, 2) + proj_slots(0, 3, "k") + vm(0, 3)
                     + proj_slots(0, 1, "q"),
                     (1, 2, 3, 4, 5, 6, 7, 8, 9, 10, 11, 12, 13)),
            (0, 1): (proj_slots(0, 2, "q") + proj_slots(1, 0, "q")
                     + proj_slots(1, 0, "k") + vm(1, 0),
                     (1, 2, 3, 4, 5, 6, 8, 10, 12)),
            (0, 2): (proj_slots(0, 3, "q") + proj_slots(1, 1, "q")
                     + proj_slots(1, 1, "k") + vm(1, 1),
                     (1, 2, 3, 4, 5, 6, 8, 10, 12)),
            (0, 3): (proj_slots(1, 2, "q") + proj_slots(1, 2, "k")
                     + vm(1, 2) + proj_slots(1, 3, "q")
                     + proj_slots(1, 3, "k") + vm(1, 3),
                     (1, 2, 3, 4, 5, 6, 7, 8, 9, 10, 12, 14)),
        }
        for p in range(NPAIRS):
            if p + 1 < NPAIRS:
                start_pair(p + 1) if p > 0 else None
            for qc in range(NQC):
                if (p, qc) in sched:
                    inter, points = sched[(p, qc)]
                elif p + 1 < NPAIRS:
                    inter = []
                    for w in ("q", "k", "v"):
                        inter += proj_slots(p + 1, qc, w)
                    points = (1, 2, 4, 6, 8, 10, 12, 14)
                else:
                    inter = []
                    if qc > 0:
                        for s in range(4):
                            inter += outproj_slots((qc - 1) * 4 + s)
                    points = (1, 2, 4, 6, 8, 10, 12, 14)
                emit_attention(p, qc, inter, points)
        for s in range(4):
            outproj_piece((NQC - 1) * 4 + s)

    nc.compile()
    return nc


def _get_compiled():
    global _COMPILED
    if _COMPILED is None:
        _COMPILED = _build()
    return _COMPILED


def make_in_maps(**inputs):
    import ml_dtypes
    bf = ml_dtypes.bfloat16
    x = np.asarray(inputs["inputs"], np.float32)
    xTb = [np.ascontiguousarray(x[b].T.astype(bf)) for b in range(B)]
    gs = {}
    for nm in ("Wq", "Wk", "Wv", "Wo", "bq", "bk", "bv"):
        a = np.asarray(inputs[nm], np.float32)
        for g in range(2):
            sl = slice(g * GCOLS, (g + 1) * GCOLS)
            if nm == "Wo":
                gs[(nm, g)] = np.ascontiguousarray(a[sl, :].astype(bf))
            elif nm.startswith("W"):
                gs[(nm, g)] = np.ascontiguousarray(a[:, sl].astype(bf))
            else:
                gs[(nm, g)] = np.ascontiguousarray(a[sl])
    in_maps = []
    for c in range(NCORES):
        g, b = c // B, c % B
        in_maps.append({
            "xT": xTb[b],
            "wq": gs[("Wq", g)], "wk": gs[("Wk", g)],
            "wv": gs[("Wv", g)], "wo": gs[("Wo", g)],
            "bq": gs[("bq", g)], "bk": gs[("bk", g)],
            "bv": gs[("bv", g)],
        })
    return in_maps


def combine(results, bo):
    out = np.empty((B, S, D), np.float32)
    bo = np.asarray(bo, np.float32)
    for b in range(B):
        out[b] = results[b]["out"] + results[B + b]["out"] + bo
    return out


def kernel(**inputs):
    from concourse import bass_utils
    nc = _get_compiled()
    in_maps = make_in_maps(**inputs)
    res = bass_utils.run_bass_kernel_spmd(
        nc, in_maps, core_ids=list(range(NCORES)))
    return combine(res.results, inputs["bo"])


# revision 4
# speedup vs baseline: 1.0801x; 1.0141x over previous
"""Multi-head attention (B=4, S=2048, D=1024, H=16) on 8 TRN2 NeuronCores. v2.

Sharding: 2D grid (batch x head-group), core c = g*4 + b.

v2 design vs baseline (544us):
- All matmul operands bf16 (host pre-converts x^T and weights to bf16;
  on-chip evictions write bf16). FWL-eligible 128-col stationaries.
- Scores for the head pair run CONCURRENTLY in 64x128 PE row tiles
  (T0: head0 on partitions 0:63, T8: head1 on 64:127) writing the two
  halves of one [128, 1024] PSUM region; ONE exp activation covers both.
- AV stationary padded to 128 cols with the ones/denominator column
  placed so head1's output lands on PSUM partitions 64:127 directly:
    h0 stationary = [V_h0(64) | ones | 0*63]   -> attn rows 0:63, den row 64
    h1 stationary = [ones | 0*63 | V_h1(64)]   -> den row 0, attn rows 64:127
  This kills the partition-shift DMA for h1 and makes every AV weight
  load a 128-col FWL load.
- ScalarE does ONLY the 256 exp instructions; all PSUM evictions and
  normalization run on DVE (tensor_scalar_add with per-partition bias).
- Software pipelining: pair p+1's projection slices and (for the last
  pair) the output projection are emitted interleaved into pair p's
  scalar-bound attention kt-loop so the PE never idles on phase edges.
PSUM budget: scores [128,1024] x2 (4 banks) + av h0/h1 (2) + proj/outproj
ping-pong (2) = 8. Transposes borrow the av-tag bank.
"""
import numpy as np

B, S, D, H, DH = 4, 2048, 1024, 16, 64
NCORES = 8
GCOLS = D // 2          # 512 cols per head-group core
NPAIRS = GCOLS // 128   # 4 head-pairs per core
NKT = S // 128          # 16 k-tiles
DC = D // 128           # 8 contraction chunks for projections
NQC = S // 512          # 4 q-chunks of 512

_COMPILED = None


def _build():
    import concourse.bass as bass
    import concourse.bacc as bacc
    import concourse.tile as tile
    from concourse import mybir
    from concourse.masks import make_identity
    from contextlib import ExitStack

    F32 = mybir.dt.float32
    BF16 = mybir.dt.bfloat16
    EXP = mybir.ActivationFunctionType.Exp

    nc = bacc.Bacc("TRN2", target_bir_lowering=False, debug=False)
    xT = nc.dram_tensor("xT", [D, S], BF16, kind="ExternalInput").ap()
    wq = nc.dram_tensor("wq", [D, GCOLS], BF16, kind="ExternalInput").ap()
    wk = nc.dram_tensor("wk", [D, GCOLS], BF16, kind="ExternalInput").ap()
    wv = nc.dram_tensor("wv", [D, GCOLS], BF16, kind="ExternalInput").ap()
    wo = nc.dram_tensor("wo", [GCOLS, D], BF16, kind="ExternalInput").ap()
    bq = nc.dram_tensor("bq", [GCOLS], F32, kind="ExternalInput").ap()
    bk = nc.dram_tensor("bk", [GCOLS], F32, kind="ExternalInput").ap()
    bv = nc.dram_tensor("bv", [GCOLS], F32, kind="ExternalInput").ap()
    out = nc.dram_tensor("out", [S, D], F32, kind="ExternalOutput").ap()

    with tile.TileContext(nc) as tc, ExitStack() as st:
        const = st.enter_context(tc.tile_pool(name="const", bufs=1))
        persist = st.enter_context(tc.tile_pool(name="persist", bufs=1))
        qkv = st.enter_context(tc.tile_pool(name="qkv", bufs=1))
        vts = st.enter_context(tc.tile_pool(name="vts", bufs=1))
        espool = st.enter_context(tc.tile_pool(name="es", bufs=1))
        small = st.enter_context(tc.tile_pool(name="small", bufs=1))
        osb = st.enter_context(tc.tile_pool(name="osb", bufs=1))
        psum = st.enter_context(tc.tile_pool(name="ps", bufs=1, space="PSUM"))

        # ---------------- constants / weights / x^T ----------------
        idf = const.tile([128, 128], F32)
        make_identity(nc, idf)
        # ones-row tile for PE den-broadcast: row 0 and row 64 are ones so
        # slices [0:64] / [64:128] each have local row 0 = ones
        ones_t = const.tile([128, 128], BF16, name="ones_t", tag="ones_t")
        nc.vector.memset(ones_t, 0.0)
        nc.vector.memset(ones_t[0:1, :], 1.0)
        nc.vector.memset(ones_t[64:65, :], 1.0)
        bias_sb = {}
        for nm, b_ap in (("q", bq), ("k", bk), ("v", bv)):
            t = const.tile([128, NPAIRS], F32, name=f"b{nm}", tag=f"b{nm}")
            nc.sync.dma_start(out=t, in_=b_ap.rearrange("(p r) -> r p", r=128))
            bias_sb[nm] = t

        xr = persist.tile([128, DC, S], BF16, name="xr", tag="xr")
        xT_d = xT.rearrange("(dc p) n -> p dc n", p=128)
        for dc in range(DC):
            [nc.sync, nc.scalar][dc % 2].dma_start(
                out=xr[:, dc, :], in_=xT_d[:, dc, :])
        w_sb = {}
        for nm, w_ap in (("q", wq), ("k", wk), ("v", wv)):
            t = persist.tile([128, DC, GCOLS], BF16, name=f"w{nm}",
                             tag=f"w{nm}")
            nc.sync.dma_start(out=t, in_=w_ap.rearrange("(dc p) m -> p dc m",
                                                        p=128))
            w_sb[nm] = t
        wo_sb = persist.tile([128, NPAIRS, D], BF16, name="wo", tag="wo")
        nc.sync.dma_start(out=wo_sb, in_=wo.rearrange("(p r) n -> r p n",
                                                      r=128))

        # v_sb: [128, 2, NKT, 256] bf16, double-buffered by pair parity;
        # per pair: cols 0:64 = V_h0, 64 = ones, 65:128 = 0,
        #           128 = ones, 129:192 = 0, 192:256 = V_h1.
        v_sb = persist.tile([128, 2, NKT, 256], BF16, name="v_sb",
                            tag="v_sb")
        nc.vector.memset(v_sb, 0.0)
        nc.vector.memset(v_sb[:, :, :, 64:65], 1.0)
        nc.vector.memset(v_sb[:, :, :, 128:129], 1.0)

        # attnT tiles (out-proj stationaries), bf16 [128, S] per pair
        attnT = [persist.tile([128, S], BF16, name=f"attnT{p}",
                              tag=f"attnT{p}") for p in range(NPAIRS)]

        # double-buffered per-pair projection outputs
        def qk_tile(nm, p):
            return qkv.tile([128, S], BF16, name=f"{nm}t", tag=f"{nm}t",
                            bufs=2)

        # ---------------- PSUM tags ----------------
        # 8 banks: sc 2x[128,1024] (4; also serves projections, V-transposes
        # and the output projection) + av 4x[128,512] (4: per-head lo/hi
        # contraction halves so AV runs in the same 64x128 tiled mode as
        # the scores -- the PE never changes tiling mode inside the
        # attention loop; mode switches cost ~240ns each).
        def sc_ps():
            return psum.tile([128, 1024], F32, name="sc", tag="sc", bufs=2)

        def av_ps():
            return psum.tile([128, 512], F32, name="av", tag="av", bufs=4)

        def proj_ps():
            t = sc_ps()
            return t[:, 0:512]

        def proj_ps2():
            return sc_ps()

        def mm(out_, l, r, **kw):
            nc.tensor.matmul(out_, l, r, skip_group_check=True, **kw)

        # ---------------- warmup (PE ramp) ----------------
        zb = const.tile([128, 512], BF16)
        nc.vector.memset(zb, 0.0)
        for i in range(24):
            w_ps = proj_ps()
            mm(w_ps, zb[:, 0:128], zb, start=True, stop=True)

        # ---------------- emission helpers ----------------
        cur = {}   # current pair's projection tiles

        def proj_slots(p, qc, which):
            """Projection nt-chunk for pair p as a list of small emission
            slots (each <= ~1.2us of PE work) so the scalar engine's 1-exp
            lookahead never runs dry while a slot occupies the PE queue."""
            nsl = slice(qc * 512, (qc + 1) * 512)
            state = {}

            def slot_a():
                state["ps"] = proj_ps()
                for dc in range(4):
                    mm(state["ps"], w_sb[which][:, dc, p * 128:(p + 1) * 128],
                       xr[:, dc, nsl], start=(dc == 0), stop=False)

            def slot_b():
                mm_ps = state["ps"]
                for dc in range(4, DC):
                    mm(mm_ps, w_sb[which][:, dc, p * 128:(p + 1) * 128],
                       xr[:, dc, nsl], start=False, stop=(dc == DC - 1))
                if which in ("q", "k"):
                    t = cur[(which, p)]
                    nc.vector.tensor_scalar_add(t[:, nsl], mm_ps,
                                                bias_sb[which][:, p:p + 1])
                else:
                    vt = vts.tile([128, 512], F32, name="vt", tag="vt",
                                  bufs=2)
                    nc.vector.tensor_scalar_add(vt, mm_ps,
                                                bias_sb["v"][:, p:p + 1])
                    state["vt"] = vt

            def slot_c():
                vt = state["vt"]
                tr = proj_ps()
                for j in range(4):
                    mm(tr[:, j * 128:(j + 1) * 128],
                       vt[:, j * 128:(j + 1) * 128], idf, is_transpose=True,
                       start=True, stop=True)
                trr = tr.rearrange("p (a b) -> p a b", b=128)
                kts = slice(qc * 4, (qc + 1) * 4)
                nc.vector.tensor_copy(v_sb[:, p % 2, kts, 0:64],
                                      trr[:, :, 0:64])
                nc.vector.tensor_copy(v_sb[:, p % 2, kts, 192:256],
                                      trr[:, :, 64:128])

            if which == "v":
                return [slot_a, slot_b, slot_c]
            return [slot_a, slot_b]

        def emit_proj_piece(p, qc, which):
            for s in proj_slots(p, qc, which):
                s()

        def start_pair(p):
            cur[("q", p)] = qk_tile("q", p)
            cur[("k", p)] = qk_tile("k", p)

        def emit_attention(p, qc, interleave, points=(1, 2, 4, 6, 8, 10,
                                                     12, 14)):
            """Attention for (pair, q-chunk); interleave = list of callables
            invoked between kt iterations (projection pieces etc.).
            Software-pipelined: scores(kt+1) is emitted BEFORE AV(kt) so the
            in-order PE queue never stalls behind the exp dependency."""
            qt, kt_t = cur[("q", p)], cur[("k", p)]
            vb = v_sb[:, p % 2]
            qsl = slice(qc * 512, (qc + 1) * 512)
            av = [av_ps() for _ in range(4)]  # h0lo h0hi h1lo h1hi
            inter = list(interleave)
            es_t = [None] * NKT

            def scores(kt):
                sc = sc_ps()
                ksl = slice(kt * 128, (kt + 1) * 128)
                mm(sc[:, 0:512], kt_t[0:64, ksl], qt[0:64, qsl],
                   start=True, stop=True)
                mm(sc[:, 512:1024], kt_t[64:128, ksl], qt[64:128, qsl],
                   start=True, stop=True)
                if kt in (6, 9, 12, 15):
                    # Schraudolph exp on DVE: bf16 bit pattern of exp(s/8)
                    # as one affine op into uint16, reinterpreted as bf16.
                    # Keeps ScalarE strictly faster than the PE so the PE
                    # never idles (idle >100ns halves the PE clock).
                    esu = espool.tile([128, 1024], mybir.dt.uint16,
                                      name="esu", tag="esu", bufs=3)
                    nc.vector.tensor_scalar(
                        out=esu, in0=sc, scalar1=23.08312065,
                        scalar2=16250.4093, op0=mybir.AluOpType.mult,
                        op1=mybir.AluOpType.add)
                    es_t[kt] = esu.bitcast(BF16)
                else:
                    es = espool.tile([128, 1024], BF16, name="es", tag="es",
                                     bufs=5)
                    nc.scalar.activation(es, sc, EXP, scale=0.125)
                    es_t[kt] = es

            scores(0)
            scores(1)
            scores(2)
            for kt in range(NKT):
                if kt + 3 < NKT:
                    scores(kt + 3)
                es = es_t[kt]
                st, sp = (kt == 0), (kt == NKT - 1)
                mm(av[0], vb[0:64, kt, 0:128], es[0:64, 0:512],
                   start=st, stop=sp)
                mm(av[1], vb[64:128, kt, 0:128], es[64:128, 0:512],
                   start=st, stop=sp)
                mm(av[2], vb[0:64, kt, 128:256], es[0:64, 512:1024],
                   start=st, stop=sp)
                mm(av[3], vb[64:128, kt, 128:256], es[64:128, 512:1024],
                   start=st, stop=sp)
                # one small slot per point, never adjacent to the window
                # boundary (a slot at kt=15 would sit in the sc-tag
                # rotation directly ahead of the next window's scores)
                if inter and kt in points:
                    inter.pop(0)()
            # merge contraction halves + normalize
            # h0: attn rows 0:63, den row 64 ; h1: attn rows 64:127, den row 0
            avh0 = small.tile([128, 512], BF16, name="avh0", tag="avh0",
                              bufs=2)
            avh1 = small.tile([128, 512], BF16, name="avh1", tag="avh1",
                              bufs=2)
            t0 = small.tile([128, 512], F32, name="tlo", tag="tlo", bufs=2)
            t1 = small.tile([128, 512], F32, name="thi", tag="thi", bufs=2)
            nc.vector.tensor_copy(t0, av[0])
            nc.vector.tensor_add(avh0, av[1], t0)
            nc.vector.tensor_copy(t1, av[2])
            nc.vector.tensor_add(avh1, av[3], t1)
            # den broadcast THROUGH THE PE: a ones-row stationary matmul in
            # the same 64x128 tile mode replicates the den row across all
            # 128 psum partitions -- no DMA, no queue contention, no mode
            # switch. bc matmuls borrow freed av-tag banks.
            bc0 = av_ps()
            bc1 = av_ps()
            mm(bc0, ones_t[64:128, :], avh0[64:128, :], start=True,
               stop=True)
            mm(bc1, ones_t[0:64, :], avh1[0:64, :], start=True, stop=True)
            # reciprocal_approx_fast requires a full-128-partition SBUF
            # source on HW (PSUM or partial-partition sources NaN silently)
            bcs = small.tile([128, 512], F32, name="bcs", tag="bcs", bufs=2)
            nc.vector.tensor_copy(bcs[0:64, :], bc0[0:64, :])
            nc.vector.tensor_copy(bcs[64:128, :], bc1[64:128, :])
            rec = small.tile([128, 512], F32, name="rec", tag="rec", bufs=2)
            nc.vector.reciprocal_approx_fast(out=rec, in_=bcs)
            nc.vector.tensor_mul(attnT[p][0:64, qsl], avh0[0:64, :],
                                 rec[0:64, :])
            nc.vector.tensor_mul(attnT[p][64:128, qsl], avh1[64:128, :],
                                 rec[64:128, :])

        def outproj_slots(qcb):
            qbs = slice(qcb * 128, (qcb + 1) * 128)
            state = {}

            def slot(nt):
                if nt == 0:
                    state["ps"] = proj_ps2()
                ops = state["ps"]
                for p in range(NPAIRS):
                    mm(ops[:, nt * 512:(nt + 1) * 512], attnT[p][:, qbs],
                       wo_sb[:, p, nt * 512:(nt + 1) * 512],
                       start=(p == 0), stop=(p == NPAIRS - 1))
                if nt == 1:
                    o_sb = osb.tile([128, 1024], F32, name="o", tag="o",
                                    bufs=3)
                    nc.vector.tensor_copy(o_sb, ops)
                    nc.sync.dma_start(out=out[qbs, :], in_=o_sb)

            return [lambda: slot(0), lambda: slot(1)]

        def outproj_piece(qcb):
            for s in outproj_slots(qcb):
                s()

        # ---------------- pipeline ----------------
        # Short prologue: only pair-0 chunk-0 projections plus k1 run before
        # the first attention window; the remaining pair-0 chunks and ALL of
        # pair-1's projections are slotted into pair-0's windows with
        # deadline-aware points (chunk j must be evicted before the scores/
        # AV instruction that reads it is EMITTED, or the in-order PE queue
        # deadlocks).
        def vm(p, qc):
            s = proj_slots(p, qc, "v")
            return [lambda a=s[0], b=s[1]: (a(), b()), s[2]]

        def km(p, qc, w):
            return [lambda pp=p, q=qc, ww=w: emit_proj_piece(pp, q, ww)]

        start_pair(0)
        for which in ("q", "k", "v"):
            emit_proj_piece(0, 0, which)
        emit_proj_piece(0, 1, "k")
        start_pair(1)
        # deadlines: k-chunk j before scores(4j) emission (kt 4j-1),
        # v-chunk j before AV(4j) emission (kt 4j), q-chunk qc before
        # window (0,qc) emission.
        sched = {
            (0, 0): (proj_slots(0, 1, "v") + km(0, 2, "k")
                     + vm(0, 2) + proj_slots(0, 3, "k") + vm(0, 3)
                     + proj_slots(0, 1, "q"),
                     (1, 2, 3, 4, 5, 6, 7, 8, 9, 10, 11, 12)),
            (0, 1): (proj_slots(0, 2, "q") + proj_slots(1, 0, "q")
                     + proj_slots(1, 0, "k") + vm(1, 0),
                     (1, 2, 3, 4, 5, 6, 8, 10, 12)),
            (0, 2): (proj_slots(0, 3, "q") + proj_slots(1, 1, "q")
                     + proj_slots(1, 1, "k") + vm(1, 1),
                     (1, 2, 3, 4, 5, 6, 8, 10, 12)),
            (0, 3): (proj_slots(1, 2, "q") + proj_slots(1, 2, "k")
                     + vm(1, 2) + proj_slots(1, 3, "q")
                     + proj_slots(1, 3, "k") + vm(1, 3),
                     (1, 2, 3, 4, 5, 6, 7, 8, 9, 10, 12, 14)),
        }
        for p in range(NPAIRS):
            if p + 1 < NPAIRS:
                start_pair(p + 1) if p > 0 else None
            for qc in range(NQC):
                if (p, qc) in sched:
                    inter, points = sched[(p, qc)]
                elif p + 1 < NPAIRS:
                    inter = []
                    for w in ("q", "k", "v"):
                        inter += proj_slots(p + 1, qc, w)
                    points = (1, 2, 4, 6, 8, 10, 12, 14)
                else:
                    inter = []
                    if qc > 0:
                        for s in range(4):
                            inter += outproj_slots((qc - 1) * 4 + s)
                    points = (1, 2, 4, 6, 8, 10, 12, 14)
                emit_attention(p, qc, inter, points)
        for s in range(4):
            outproj_piece((NQC - 1) * 4 + s)

    nc.compile()
    return nc


def _get_compiled():
    global _COMPILED
    if _COMPILED is None:
        _COMPILED = _build()
    return _COMPILED


def make_in_maps(**inputs):
    import ml_dtypes
    bf = ml_dtypes.bfloat16
    x = np.asarray(inputs["inputs"], np.float32)
    xTb = [np.ascontiguousarray(x[b].T.astype(bf)) for b in range(B)]
    gs = {}
    for nm in ("Wq", "Wk", "Wv", "Wo", "bq", "bk", "bv"):
        a = np.asarray(inputs[nm], np.float32)
        for g in range(2):
            sl = slice(g * GCOLS, (g + 1) * GCOLS)
            if nm == "Wo":
                gs[(nm, g)] = np.ascontiguousarray(a[sl, :].astype(bf))
            elif nm.startswith("W"):
                gs[(nm, g)] = np.ascontiguousarray(a[:, sl].astype(bf))
            else:
                gs[(nm, g)] = np.ascontiguousarray(a[sl])
    in_maps = []
    for c in range(NCORES):
        g, b = c // B, c % B
        in_maps.append({
            "xT": xTb[b],
            "wq": gs[("Wq", g)], "wk": gs[("Wk", g)],
            "wv": gs[("Wv", g)], "wo": gs[("Wo", g)],
            "bq": gs[("bq", g)], "bk": gs[("bk", g)],
            "bv": gs[("bv", g)],
        })
    return in_maps


def combine(results, bo):
    out = np.empty((B, S, D), np.float32)
    bo = np.asarray(bo, np.float32)
    for b in range(B):
        out[b] = results[b]["out"] + results[B + b]["out"] + bo
    return out


def kernel(**inputs):
    from concourse import bass_utils
    nc = _get_compiled()
    in_maps = make_in_maps(**inputs)
    res = bass_utils.run_bass_kernel_spmd(
        nc, in_maps, core_ids=list(range(NCORES)))
    return combine(res.results, inputs["bo"])
